# revision 12
# baseline (speedup 1.0000x reference)
"""Trainium2 Bass kernel for one DNC memory-addressing timestep.

Contract: kernel(**inputs) takes the FULL (unsharded) numpy inputs of
reference.setup_inputs() and returns the full outputs
(read_weights, write_weights, usage, link, precedence) as numpy float32.

Sharding: pure data parallel over batch dim 0 across 8 NeuronCores
(8 batch elements per core, no cross-core communication).

Algorithm notes (per batch element, N=1024 slots, W=64 word, R=4 reads):
  - usage / precedence / write weights: exact elementwise math, batched
    across the 8 per-core elements on partitions 0..7 ("row layout"
    [8, 1024]) so every vector op costs one instruction per core.
  - allocation: the reference sorts usage; here we use the equivalent
    closed form alloc[i] = nonusage[i] * prod_{j: u_j < u_i} u_j
                        = nonusage[i] * exp(sum_j [u_j < u_i] * log u_j)
    computed with comparison tiles (DVE is_gt) contracted against
    log(u) columns on the TensorEngine.  (fp32 ties have ~0 probability
    and were validated against the reference to 3e-8 absmax.)
  - cosine content addressing: dot products via PE after on-chip
    transposes of memory; softmax batched over all 8*5 head-rows.
  - link update (the memory-bound bulk: 8MB/elt of HBM traffic) is two
    fused scalar_tensor_tensor ops per [128, 1024] tile:
        out = (a_i - w_j) * L         (a = 1 - w, per-partition scalar)
        out = (p_j * w_i) + out
    plus a [128,128] diagonal mask multiply, split DVE/GPSIMD.
"""

import os
import sys
from contextlib import ExitStack

for _p in ("/opt/trn_rl_repo", "/root/.axon_site/_ro/trn_rl_repo"):
    if os.path.isdir(_p) and _p not in sys.path:
        sys.path.append(_p)

import numpy as np

import concourse.bass as bass
import concourse.bacc as bacc
import concourse.mybir as mybir
import concourse.tile as tile
from concourse.bass_utils import run_bass_kernel_spmd

F32 = mybir.dt.float32
AF = mybir.ActivationFunctionType
OP = mybir.AluOpType
AX = mybir.AxisListType
ts = bass.ts

EPS = 1e-5
B, N, W, R, NW = 64, 1024, 64, 4, 1
NCORES = 8
E = B // NCORES          # batch elements per core = 8
T = N // 128             # 128-row chunks per link matrix = 8
H = R + NW               # stacked heads (4 read + 1 write) = 5

# How many of the 8 link-update "first op" tiles per element run on
# GPSIMD instead of DVE (load balancing knob).
GPS_STT = int(os.environ.get("K_GPS_STT", "3"))
# How many of the 8 allocation compare tiles per element run on GPSIMD.
GPS_CMP = int(os.environ.get("K_GPS_CMP", "2"))
# Engine for the diagonal mask multiply: "pool" or "dve".
DIAG_ENG = os.environ.get("K_DIAG_ENG", "pool")

TRACE = False
TRACE_KW = {}
LAST_RESULTS = None

_NC = None


def _emit(nc, tc, io):
    (mem_d, rk_d, rs_d, wk_d, ws_d, fg_d, ag_d, wg_d, prw_d, pww_d, pu_d,
     pl_d, pp_d, ident_d, dmask_d, sel8_d, selE4_d, selM_d,
     orw_d, oww_d, ou_d, ol_d, op_d) = io

    ctx = ExitStack()
    cpool = ctx.enter_context(tc.tile_pool(name="consts", bufs=1))
    gpool = ctx.enter_context(tc.tile_pool(name="glob", bufs=1))
    tpool = ctx.enter_context(tc.tile_pool(name="tmp", bufs=5))
    epool = ctx.enter_context(tc.tile_pool(name="pere", bufs=2))
    e1pool = ctx.enter_context(tc.tile_pool(name="pere1", bufs=1))
    ctpool = ctx.enter_context(tc.tile_pool(name="ct", bufs=3))
    lpool = ctx.enter_context(tc.tile_pool(name="lin", bufs=7))
    cgpool = ctx.enter_context(tc.tile_pool(name="cg", bufs=2))
    opool = ctx.enter_context(tc.tile_pool(name="lout", bufs=3))
    ps_big = ctx.enter_context(tc.tile_pool(name="ps_big", bufs=1, space="PSUM"))
    ps_mid = ctx.enter_context(tc.tile_pool(name="ps_mid", bufs=1, space="PSUM"))
    ps_acc = ctx.enter_context(tc.tile_pool(name="ps_acc", bufs=2, space="PSUM"))
    ps_tr = ctx.enter_context(tc.tile_pool(name="ps_tr", bufs=2, space="PSUM"))

    # ---- constants ----
    ident = cpool.tile([128, 128], F32, tag="ident")
    dmask = cpool.tile([128, 128], F32, tag="dmask")
    sel8 = cpool.tile([8, 1024], F32, tag="sel8")
    selE4 = cpool.tile([32, 8], F32, tag="selE4")
    selM = cpool.tile([8, 40], F32, tag="selM")
    ones64 = cpool.tile([64, 1], F32, tag="ones64")
    epsv = cpool.tile([128, 1], F32, tag="epsv")
    nc.sync.dma_start(ident[:], ident_d[:])
    nc.sync.dma_start(dmask[:], dmask_d[:])
    nc.sync.dma_start(sel8[:], sel8_d[:])
    nc.sync.dma_start(selE4[:], selE4_d[:])
    nc.sync.dma_start(selM[:], selM_d[:])
    nc.vector.memset(ones64[:], 1.0)
    nc.vector.memset(epsv[:], EPS)

    # ---- small input loads ----
    prw_all = gpool.tile([32, 1024], F32, tag="prw")       # rows 4e+r
    fg_all = gpool.tile([32, 1], F32, tag="fg")
    pu8 = gpool.tile([8, 1024], F32, tag="pu8")
    pww8 = gpool.tile([8, 1024], F32, tag="pww8")
    pp8 = gpool.tile([8, 1024], F32, tag="pp8")
    ag8 = gpool.tile([8, 1], F32, tag="ag8")
    wg8 = gpool.tile([8, 1], F32, tag="wg8")
    K40 = gpool.tile([40, 64], F32, tag="K40")             # rows 5e+h
    st40 = gpool.tile([40, 1], F32, tag="st40")
    nc.sync.dma_start(prw_all[:], prw_d[:, :, :].rearrange("e r n -> (e r) n"))
    nc.sync.dma_start(fg_all[:, 0], fg_d[:, :].rearrange("e r -> (e r)"))
    nc.sync.dma_start(pu8[:], pu_d[:, :])
    nc.sync.dma_start(pww8[:], pww_d[:, 0, :])
    nc.sync.dma_start(pp8[:], pp_d[:, 0, :])
    nc.sync.dma_start(ag8[:], ag_d[:, :])
    nc.sync.dma_start(wg8[:], wg_d[:, :])
    for e in range(E):
        nc.sync.dma_start(K40[e * H:e * H + R, :], rk_d[e, :, :])
        nc.sync.dma_start(K40[e * H + R:e * H + H, :], wk_d[e, :, :])
        nc.sync.dma_start(st40[e * H:e * H + R, 0], rs_d[e, :])
        nc.sync.dma_start(st40[e * H + R:e * H + H, 0], ws_d[e, :])

    # ---- phase 1: usage (batched rows [8, 1024]) ----
    negfg = gpool.tile([32, 1], F32, tag="negfg")
    nc.scalar.mul(negfg[:], fg_all[:], -1.0)
    t_all = tpool.tile([32, 1024], F32, tag="tmp40", name="t_all")
    nc.scalar.activation(t_all[:], prw_all[:], AF.Identity, bias=1.0, scale=negfg[:])
    lt_all = tpool.tile([32, 1024], F32, tag="tmp40", name="lt_all")
    nc.scalar.activation(lt_all[:], t_all[:], AF.Ln)
    psum_phi = ps_mid.tile([40, 1024], F32, tag="pmid")
    for h in range(2):
        nc.tensor.matmul(psum_phi[0:8, ts(h, 512)], selE4[:, :], lt_all[:, ts(h, 512)],
                         start=True, stop=True)
    phi8 = tpool.tile([8, 1024], F32, tag="tmp40", name="phi8")
    nc.scalar.activation(phi8[:], psum_phi[0:8, :], AF.Exp)

    onem_pu8 = tpool.tile([8, 1024], F32, tag="tmp40", name="onem_pu8")
    nc.scalar.activation(onem_pu8[:], pu8[:], AF.Identity, bias=1.0, scale=-1.0)
    onem_pww8 = tpool.tile([8, 1024], F32, tag="tmp40", name="onem_pww8")
    nc.scalar.activation(onem_pww8[:], pww8[:], AF.Identity, bias=1.0, scale=-1.0)
    mm2 = tpool.tile([8, 1024], F32, tag="tmp40", name="mm2")
    nc.vector.tensor_mul(mm2[:], onem_pu8[:], onem_pww8[:])
    u18 = tpool.tile([8, 1024], F32, tag="tmp40", name="u18")
    nc.scalar.activation(u18[:], mm2[:], AF.Identity, bias=1.0, scale=-1.0)
    usage8 = tpool.tile([8, 1024], F32, tag="tmp40", name="usage8")
    nc.vector.tensor_mul(usage8[:], u18[:], phi8[:])
    nc.sync.dma_start(ou_d[:, :], usage8[:])

    ua8 = gpool.tile([8, 1024], F32, tag="ua8")
    nc.scalar.activation(ua8[:], usage8[:], AF.Identity, bias=epsv[0:8, :], scale=1.0 - EPS)
    lua8 = tpool.tile([8, 1024], F32, tag="tmp40", name="lua8")
    nc.scalar.activation(lua8[:], ua8[:], AF.Ln)
    nonu8 = gpool.tile([8, 1024], F32, tag="nonu8")
    nc.scalar.activation(nonu8[:], ua8[:], AF.Identity, bias=1.0, scale=-1.0)

    # ---- phase 2: column layouts of ua / log(ua) : [128, 8*T] ----
    ucol = gpool.tile([128, 64], F32, tag="ucol")   # [:, t*8+e]
    lcol = gpool.tile([128, 64], F32, tag="lcol")
    for t in range(T):
        p1 = ps_tr.tile([128, 8], F32, tag="tr")
        nc.tensor.transpose(p1[:], ua8[:, ts(t, 128)], ident[0:8, 0:8])
        nc.scalar.copy(ucol[:, ts(t, 8)], p1[:])
        p2 = ps_tr.tile([128, 8], F32, tag="tr")
        nc.tensor.transpose(p2[:], lua8[:, ts(t, 128)], ident[0:8, 0:8])
        nc.scalar.copy(lcol[:, ts(t, 8)], p2[:])

    # ---- phase 3: allocation weights per element ----
    es8 = gpool.tile([8, 1024], F32, tag="es8")
    for e in range(E):
        psum_ub = ps_big.tile([128, 1024], F32, tag="pbig")
        for h in range(2):
            nc.tensor.matmul(psum_ub[:, ts(h, 512)], sel8[:, ts(e, 128)],
                             ua8[:, ts(h, 512)], start=True, stop=True)
        ub_sb = epool.tile([128, 1024], F32, tag="ub_sb")
        nc.scalar.copy(ub_sb[:], psum_ub[:])

        ps_s = [ps_acc.tile([1, 512], F32, tag="pacc", name=f"ps_s{h}") for h in range(2)]
        for t in range(T):
            ct = ctpool.tile([128, 1024], F32, tag="CT")
            eng = nc.gpsimd if t < GPS_CMP else nc.vector
            eng.tensor_scalar(ct[:], ub_sb[:], ucol[:, t * 8 + e: t * 8 + e + 1],
                              None, OP.is_gt)
            for h in range(2):
                nc.tensor.matmul(ps_s[h][:, :], lcol[:, t * 8 + e: t * 8 + e + 1],
                                 ct[:, ts(h, 512)], start=(t == 0), stop=(t == T - 1))
        es_e = e1pool.tile([1, 1024], F32, tag="es_e")
        for h in range(2):
            nc.scalar.activation(es_e[:, ts(h, 512)], ps_s[h][:, :], AF.Exp)
        nc.sync.dma_start(es8[e:e + 1, :], es_e[:])

    alloc8 = gpool.tile([8, 1024], F32, tag="alloc8")
    nc.vector.tensor_mul(alloc8[:], nonu8[:], es8[:])

    # ---- phase 4: cosine content weights ----
    # keys: transpose [40, 64] -> [64, 40]
    p_k = ps_tr.tile([64, 40], F32, tag="tr")
    nc.tensor.transpose(p_k[:], K40[:], ident[0:40, 0:40])
    k5t = gpool.tile([64, 40], F32, tag="k5t")
    nc.scalar.copy(k5t[:], p_k[:])
    # key norms and softplus(strengths)
    sqk = gpool.tile([40, 64], F32, tag="sqk")
    nc.scalar.activation(sqk[:], K40[:], AF.Square)
    kn2 = gpool.tile([40, 1], F32, tag="kn2")
    nc.vector.reduce_sum(kn2[:], sqk[:], axis=AX.X)
    kn40 = gpool.tile([40, 1], F32, tag="kn40")
    nc.scalar.activation(kn40[:], kn2[:], AF.Sqrt, bias=epsv[0:40, :])
    e40 = gpool.tile([40, 1], F32, tag="e40")
    nc.scalar.activation(e40[:], st40[:], AF.Exp)
    sp40 = gpool.tile([40, 1], F32, tag="sp40")
    nc.scalar.activation(sp40[:], e40[:], AF.Ln, bias=1.0)

    mn8 = gpool.tile([8, 1024], F32, tag="mn8")
    d40 = gpool.tile([40, 1024], F32, tag="d40")
    for e in range(E):
        mem_e = epool.tile([128, 512], F32, tag="mem_e")
        nc.sync.dma_start(
            mem_e[:].rearrange("p (t w) -> p t w", w=W),
            mem_d[e].rearrange("(t p) w -> p t w", p=128),
        )
        memt = e1pool.tile([64, 1024], F32, tag="memt")
        for t in range(T):
            p3 = ps_tr.tile([64, 128], F32, tag="tr")
            nc.tensor.transpose(p3[:], mem_e[:, ts(t, W)], ident[:, :])
            nc.scalar.copy(memt[:, ts(t, 128)], p3[:])
        sq_e = e1pool.tile([64, 1024], F32, tag="sq_e")
        nc.scalar.activation(sq_e[:], memt[:], AF.Square)
        ps_mn = [ps_acc.tile([1, 512], F32, tag="pacc", name=f"ps_mn{h}") for h in range(2)]
        for h in range(2):
            nc.tensor.matmul(ps_mn[h][:, :], ones64[:, :], sq_e[:, ts(h, 512)],
                             start=True, stop=True)
        mn_e = e1pool.tile([1, 1024], F32, tag="mn_e")
        for h in range(2):
            nc.scalar.activation(mn_e[:, ts(h, 512)], ps_mn[h][:, :], AF.Sqrt, bias=epsv[0:1, :])
        nc.sync.dma_start(mn8[e:e + 1, :], mn_e[:])

        ps_d5 = ps_mid.tile([40, 1024], F32, tag="pmid")
        for h in range(2):
            nc.tensor.matmul(ps_d5[0:H, ts(h, 512)], k5t[:, ts(e, H)],
                             memt[:, ts(h, 512)], start=True, stop=True)
        d5sb = e1pool.tile([5, 1024], F32, tag="d5sb")
        nc.scalar.copy(d5sb[:], ps_d5[0:H, :])
        nc.sync.dma_start(d40[e * H:(e + 1) * H, :], d5sb[:])

    # norms denominator: mn40[5e+h, n] = mn8[e, n]
    ps_mn40 = ps_mid.tile([40, 1024], F32, tag="pmid")
    for h in range(2):
        nc.tensor.matmul(ps_mn40[:, ts(h, 512)], selM[:, :], mn8[:, ts(h, 512)],
                         start=True, stop=True)
    nd40 = tpool.tile([40, 1024], F32, tag="tmp40", name="nd40")
    nc.scalar.activation(nd40[:], ps_mn40[:, :], AF.Identity, bias=epsv[0:40, :], scale=kn40[:])
    rec40 = tpool.tile([40, 1024], F32, tag="tmp40", name="rec40")
    nc.vector.reciprocal(rec40[:], nd40[:])
    recsp40 = tpool.tile([40, 1024], F32, tag="tmp40", name="recsp40")
    nc.vector.tensor_scalar(recsp40[:], rec40[:], sp40[:], None, OP.mult)
    sharp40 = tpool.tile([40, 1024], F32, tag="tmp40", name="sharp40")
    nc.vector.tensor_mul(sharp40[:], d40[:], recsp40[:])

    # softmax over n (batched over all 40 head-rows)
    mx40 = gpool.tile([40, 1], F32, tag="mx40")
    nc.vector.reduce_max(mx40[:], sharp40[:], axis=AX.X)
    negmx = gpool.tile([40, 1], F32, tag="negmx")
    nc.scalar.mul(negmx[:], mx40[:], -1.0)
    ex40 = tpool.tile([40, 1024], F32, tag="tmp40", name="ex40")
    sumex = gpool.tile([40, 1], F32, tag="sumex")
    nc.scalar.activation(ex40[:], sharp40[:], AF.Exp, bias=negmx[:], accum_out=sumex[:])
    inv40 = gpool.tile([40, 1], F32, tag="inv40")
    nc.vector.reciprocal(inv40[:], sumex[:])
    sm40 = gpool.tile([40, 1024], F32, tag="sm40")
    nc.vector.tensor_scalar(sm40[:], ex40[:], inv40[:], None, OP.mult)
    wc8 = gpool.tile([8, 1024], F32, tag="wc8")
    for e in range(E):
        nc.sync.dma_start(orw_d[e, :, :], sm40[e * H:e * H + R, :])
        nc.sync.dma_start(wc8[e:e + 1, :], sm40[e * H + R:e * H + H, :])

    # ---- phase 5: write weights + precedence (batched [8, 1024]) ----
    onem_ag8 = gpool.tile([8, 1], F32, tag="onem_ag8")
    nc.scalar.activation(onem_ag8[:], ag8[:], AF.Identity, bias=1.0, scale=-1.0)
    c1_8 = gpool.tile([8, 1], F32, tag="c1_8")
    nc.vector.tensor_mul(c1_8[:], wg8[:], ag8[:])
    c2_8 = gpool.tile([8, 1], F32, tag="c2_8")
    nc.vector.tensor_mul(c2_8[:], wg8[:], onem_ag8[:])
    wpart8 = gpool.tile([8, 1024], F32, tag="wpart8")
    nc.vector.tensor_scalar(wpart8[:], wc8[:], c2_8[:], None, OP.mult)
    w8 = gpool.tile([8, 1024], F32, tag="w8")
    wsum8 = gpool.tile([8, 1], F32, tag="wsum8")
    nc.vector.scalar_tensor_tensor(w8[:], alloc8[:], c1_8[:], wpart8[:],
                                   OP.mult, OP.add, accum_out=wsum8[:])
    nc.sync.dma_start(oww_d[:, 0, :], w8[:])
    onem_ws8 = gpool.tile([8, 1], F32, tag="onem_ws8")
    nc.scalar.activation(onem_ws8[:], wsum8[:], AF.Identity, bias=1.0, scale=-1.0)
    prec8 = gpool.tile([8, 1024], F32, tag="prec8")
    nc.vector.scalar_tensor_tensor(prec8[:], pp8[:], onem_ws8[:], w8[:],
                                   OP.mult, OP.add)
    nc.sync.dma_start(op_d[:, 0, :], prec8[:])

    # w columns for the link update: wcol[:, t*8+e] = w8[e, t*128+p]
    wcol = gpool.tile([128, 64], F32, tag="wcol")
    for t in range(T):
        p4 = ps_tr.tile([128, 8], F32, tag="tr")
        nc.tensor.transpose(p4[:], w8[:, ts(t, 128)], ident[0:8, 0:8])
        nc.scalar.copy(wcol[:, ts(t, 8)], p4[:])
    acol = gpool.tile([128, 64], F32, tag="acol")
    nc.scalar.activation(acol[:], wcol[:], AF.Identity, bias=1.0, scale=-1.0)

    # ---- phase 6: link update (the bulk) ----
    for e in range(E):
        psum_wb = ps_big.tile([128, 1024], F32, tag="pbig")
        for h in range(2):
            nc.tensor.matmul(psum_wb[:, ts(h, 512)], sel8[:, ts(e, 128)],
                             w8[:, ts(h, 512)], start=True, stop=True)
        wn_e = epool.tile([128, 1024], F32, tag="wn_e")
        nc.scalar.mul(wn_e[:], psum_wb[:], -1.0)

        psum_pb = ps_big.tile([128, 1024], F32, tag="pbig")
        for h in range(2):
            nc.tensor.matmul(psum_pb[:, ts(h, 512)], sel8[:, ts(e, 128)],
                             pp8[:, ts(h, 512)], start=True, stop=True)
        pb_e = epool.tile([128, 1024], F32, tag="pb_e")
        nc.scalar.copy(pb_e[:], psum_pb[:])

        for t in range(T):
            lt = lpool.tile([128, 1024], F32, tag="L")
            nc.sync.dma_start(lt[:], pl_d[e, 0, ts(t, 128), :])
            ot = opool.tile([128, 1024], F32, tag="O")
            a_sc = acol[:, t * 8 + e: t * 8 + e + 1]
            w_sc = wcol[:, t * 8 + e: t * 8 + e + 1]
            if t < GPS_STT:
                cgen = cgpool.tile([128, 1024], F32, tag="cgen", name="cgen")
                nc.scalar.activation(cgen[:], wn_e[:], AF.Identity, bias=a_sc)
                nc.gpsimd.tensor_mul(ot[:], cgen[:], lt[:])
            else:
                nc.vector.scalar_tensor_tensor(ot[:], wn_e[:], a_sc, lt[:],
                                               OP.add, OP.mult)
            nc.vector.scalar_tensor_tensor(ot[:], pb_e[:], w_sc, ot[:],
                                           OP.mult, OP.add)
            diag_eng = nc.gpsimd if DIAG_ENG == "pool" else nc.vector
            diag_eng.tensor_mul(ot[:, ts(t, 128)], ot[:, ts(t, 128)], dmask[:])
            nc.sync.dma_start(ol_d[e, 0, ts(t, 128), :], ot[:])

    ctx.close()


def _build():
    global _NC
    if _NC is not None:
        return _NC
    nc = bacc.Bacc("TRN2", target_bir_lowering=False, debug=False, num_devices=NCORES)
    mem_d = nc.dram_tensor("memory", [E, N, W], F32, kind="ExternalInput")
    rk_d = nc.dram_tensor("read_keys", [E, R, W], F32, kind="ExternalInput")
    rs_d = nc.dram_tensor("read_strengths", [E, R], F32, kind="ExternalInput")
    wk_d = nc.dram_tensor("write_keys", [E, NW, W], F32, kind="ExternalInput")
    ws_d = nc.dram_tensor("write_strengths", [E, NW], F32, kind="ExternalInput")
    fg_d = nc.dram_tensor("free_gate", [E, R], F32, kind="ExternalInput")
    ag_d = nc.dram_tensor("alloc_gate", [E, NW], F32, kind="ExternalInput")
    wg_d = nc.dram_tensor("write_gate", [E, NW], F32, kind="ExternalInput")
    prw_d = nc.dram_tensor("prev_read_weights", [E, R, N], F32, kind="ExternalInput")
    pww_d = nc.dram_tensor("prev_write_weights", [E, NW, N], F32, kind="ExternalInput")
    pu_d = nc.dram_tensor("prev_usage", [E, N], F32, kind="ExternalInput")
    pl_d = nc.dram_tensor("prev_link", [E, NW, N, N], F32, kind="ExternalInput")
    pp_d = nc.dram_tensor("prev_precedence", [E, NW, N], F32, kind="ExternalInput")
    ident_d = nc.dram_tensor("c_ident", [128, 128], F32, kind="ExternalInput")
    dmask_d = nc.dram_tensor("c_dmask", [128, 128], F32, kind="ExternalInput")
    sel8_d = nc.dram_tensor("c_sel8", [8, 1024], F32, kind="ExternalInput")
    selE4_d = nc.dram_tensor("c_selE4", [32, 8], F32, kind="ExternalInput")
    selM_d = nc.dram_tensor("c_selM", [8, 40], F32, kind="ExternalInput")
    orw_d = nc.dram_tensor("o_read_weights", [E, R, N], F32, kind="ExternalOutput")
    oww_d = nc.dram_tensor("o_write_weights", [E, NW, N], F32, kind="ExternalOutput")
    ou_d = nc.dram_tensor("o_usage", [E, N], F32, kind="ExternalOutput")
    ol_d = nc.dram_tensor("o_link", [E, NW, N, N], F32, kind="ExternalOutput")
    op_d = nc.dram_tensor("o_precedence", [E, NW, N], F32, kind="ExternalOutput")
    io = (mem_d, rk_d, rs_d, wk_d, ws_d, fg_d, ag_d, wg_d, prw_d, pww_d, pu_d,
          pl_d, pp_d, ident_d, dmask_d, sel8_d, selE4_d, selM_d,
          orw_d, oww_d, ou_d, ol_d, op_d)
    with tile.TileContext(nc) as tc:
        _emit(nc, tc, io)
    nc.compile()
    _NC = nc
    return nc


def _consts():
    eye = np.eye(128, dtype=np.float32)
    return {
        "c_ident": eye,
        "c_dmask": (1.0 - eye).astype(np.float32),
        "c_sel8": np.repeat(np.eye(8, dtype=np.float32), 128, axis=1),
        "c_selE4": np.repeat(np.eye(8, dtype=np.float32), 4, axis=0),
        "c_selM": np.repeat(np.eye(8, dtype=np.float32), 5, axis=1),
    }


def kernel(memory, read_keys, read_strengths, write_keys, write_strengths,
           free_gate, alloc_gate, write_gate, prev_read_weights,
           prev_write_weights, prev_usage, prev_link, prev_precedence):
    global LAST_RESULTS
    nc = _build()
    full = {
        "memory": memory, "read_keys": read_keys,
        "read_strengths": read_strengths, "write_keys": write_keys,
        "write_strengths": write_strengths, "free_gate": free_gate,
        "alloc_gate": alloc_gate, "write_gate": write_gate,
        "prev_read_weights": prev_read_weights,
        "prev_write_weights": prev_write_weights, "prev_usage": prev_usage,
        "prev_link": prev_link, "prev_precedence": prev_precedence,
    }
    consts = _consts()
    in_maps = []
    for c in range(NCORES):
        m = {k: np.ascontiguousarray(np.asarray(v)[c * E:(c + 1) * E],
                                     dtype=np.float32)
             for k, v in full.items()}
        m.update(consts)
        in_maps.append(m)
    res = run_bass_kernel_spmd(nc, in_maps, core_ids=list(range(NCORES)),
                               trace=TRACE, **TRACE_KW)
    LAST_RESULTS = res
    outs = res.results
    read_weights = np.concatenate([outs[c]["o_read_weights"] for c in range(NCORES)], 0)
    write_weights = np.concatenate([outs[c]["o_write_weights"] for c in range(NCORES)], 0)
    usage = np.concatenate([outs[c]["o_usage"] for c in range(NCORES)], 0)
    link = np.concatenate([outs[c]["o_link"] for c in range(NCORES)], 0)
    precedence = np.concatenate([outs[c]["o_precedence"] for c in range(NCORES)], 0)
    return (read_weights, write_weights, usage, link, precedence)


# revision 17
# speedup vs baseline: 1.4173x; 1.4173x over previous
"""Trainium2 Bass kernel for one DNC memory-addressing timestep.

Contract: kernel(**inputs) takes the FULL (unsharded) numpy inputs of
reference.setup_inputs() and returns the full outputs
(read_weights, write_weights, usage, link, precedence) as numpy float32.

Sharding: pure data parallel over batch dim 0 across 8 NeuronCores
(8 batch elements per core, no cross-core communication).

Algorithm notes (per batch element, N=1024 slots, W=64 word, R=4 reads):
  - usage / precedence / write weights: exact elementwise math, batched
    across the 8 per-core elements on partitions 0..7 ("row layout"
    [8, 1024]) so every vector op costs one instruction per core.
  - allocation: the reference sorts usage; here we use the equivalent
    closed form alloc[i] = nonusage[i] * prod_{j: u_j < u_i} u_j
                        = nonusage[i] * exp(sum_j [u_j < u_i] * log u_j)
    computed with comparison tiles (DVE is_gt) contracted against
    log(u) columns on the TensorEngine.  (fp32 ties have ~0 probability
    and were validated against the reference to 3e-8 absmax.)
  - cosine content addressing: dot products via PE after on-chip
    transposes of memory; softmax batched over all 8*5 head-rows.
  - link update (the memory-bound bulk: 8MB/elt of HBM traffic) is two
    fused scalar_tensor_tensor ops per [128, 1024] tile:
        out = (a_i - w_j) * L         (a = 1 - w, per-partition scalar)
        out = (p_j * w_i) + out
    plus a [128,128] diagonal mask multiply, split DVE/GPSIMD.
"""

import os
import sys
from contextlib import ExitStack

for _p in ("/opt/trn_rl_repo", "/root/.axon_site/_ro/trn_rl_repo"):
    if os.path.isdir(_p) and _p not in sys.path:
        sys.path.append(_p)

import numpy as np

import concourse.bass as bass
import concourse.bacc as bacc
import concourse.mybir as mybir
import concourse.tile as tile
from concourse.bass_utils import run_bass_kernel_spmd

F32 = mybir.dt.float32
AF = mybir.ActivationFunctionType
OP = mybir.AluOpType
AX = mybir.AxisListType
ts = bass.ts

EPS = 1e-5
B, N, W, R, NW = 64, 1024, 64, 4, 1
NCORES = 8
E = B // NCORES          # batch elements per core = 8
T = N // 128             # 128-row chunks per link matrix = 8
H = R + NW               # stacked heads (4 read + 1 write) = 5

# How many of the 8 link-update "first op" tiles per element run on
# GPSIMD instead of DVE (load balancing knob).
GPS_STT = int(os.environ.get("K_GPS_STT", "0"))
# How many of the 8 allocation compare tiles per element run on GPSIMD.
GPS_CMP = int(os.environ.get("K_GPS_CMP", "0"))
# Engine for the diagonal mask multiply: "pool" or "dve".
DIAG_ENG = os.environ.get("K_DIAG_ENG", "pool")
# Compare implementation: "act" = Sign(u_i - u_j) on ScalarE, "dve" = is_gt.
CMP_IMPL = os.environ.get("K_CMP_IMPL", "act")

TRACE = False
TRACE_KW = {}
LAST_RESULTS = None

_NC = None


def _emit(nc, tc, io):
    (mem_d, rk_d, rs_d, wk_d, ws_d, fg_d, ag_d, wg_d, prw_d, pww_d, pu_d,
     pl_d, pp_d, ident_d, dmask_d, sel8_d, selE4_d, selM_d,
     orw_d, oww_d, ou_d, ol_d, op_d) = io

    ctx = ExitStack()
    cpool = ctx.enter_context(tc.tile_pool(name="consts", bufs=1))
    gpool = ctx.enter_context(tc.tile_pool(name="glob", bufs=1))
    tpool = ctx.enter_context(tc.tile_pool(name="tmp", bufs=5))
    epool = ctx.enter_context(tc.tile_pool(name="pere", bufs=2))
    e1pool = ctx.enter_context(tc.tile_pool(name="pere1", bufs=1))
    ctpool = ctx.enter_context(tc.tile_pool(name="ct", bufs=3))
    lpool = ctx.enter_context(tc.tile_pool(name="lin", bufs=7))
    cgpool = ctx.enter_context(tc.tile_pool(name="cg", bufs=2))
    opool = ctx.enter_context(tc.tile_pool(name="lout", bufs=3))
    ps_big = ctx.enter_context(tc.tile_pool(name="ps_big", bufs=1, space="PSUM"))
    ps_acc = ctx.enter_context(tc.tile_pool(name="ps_acc", bufs=4, space="PSUM"))
    ps_tr = ctx.enter_context(tc.tile_pool(name="ps_tr", bufs=2, space="PSUM"))

    # ---- constants ----
    ident = cpool.tile([128, 128], F32, tag="ident")
    dmask = cpool.tile([128, 128], F32, tag="dmask")
    sel8 = cpool.tile([8, 1024], F32, tag="sel8")
    selE4 = cpool.tile([32, 8], F32, tag="selE4")
    selM = cpool.tile([8, 40], F32, tag="selM")
    ones64 = cpool.tile([64, 1], F32, tag="ones64")
    epsv = cpool.tile([128, 1], F32, tag="epsv")
    nc.sync.dma_start(ident[:], ident_d[:])
    nc.sync.dma_start(dmask[:], dmask_d[:])
    nc.sync.dma_start(sel8[:], sel8_d[:])
    nc.sync.dma_start(selE4[:], selE4_d[:])
    nc.sync.dma_start(selM[:], selM_d[:])
    nc.vector.memset(ones64[:], 1.0)
    nc.vector.memset(epsv[:], EPS)

    # ---- small input loads ----
    prw_all = gpool.tile([32, 1024], F32, tag="prw")       # rows 4e+r
    fg_all = gpool.tile([32, 1], F32, tag="fg")
    pu8 = gpool.tile([8, 1024], F32, tag="pu8")
    pww8 = gpool.tile([8, 1024], F32, tag="pww8")
    pp8 = gpool.tile([8, 1024], F32, tag="pp8")
    ag8 = gpool.tile([8, 1], F32, tag="ag8")
    wg8 = gpool.tile([8, 1], F32, tag="wg8")
    K40 = gpool.tile([40, 64], F32, tag="K40")             # rows 5e+h
    st40 = gpool.tile([40, 1], F32, tag="st40")
    nc.sync.dma_start(prw_all[:], prw_d[:, :, :].rearrange("e r n -> (e r) n"))
    nc.sync.dma_start(fg_all[:, 0], fg_d[:, :].rearrange("e r -> (e r)"))
    nc.sync.dma_start(pu8[:], pu_d[:, :])
    nc.sync.dma_start(pww8[:], pww_d[:, 0, :])
    nc.sync.dma_start(pp8[:], pp_d[:, 0, :])
    nc.sync.dma_start(ag8[:], ag_d[:, :])
    nc.sync.dma_start(wg8[:], wg_d[:, :])
    for e in range(E):
        nc.sync.dma_start(K40[e * H:e * H + R, :], rk_d[e, :, :])
        nc.sync.dma_start(K40[e * H + R:e * H + H, :], wk_d[e, :, :])
        nc.sync.dma_start(st40[e * H:e * H + R, 0], rs_d[e, :])
        nc.sync.dma_start(st40[e * H + R:e * H + H, 0], ws_d[e, :])

    # ---- phase 1: usage (batched rows [8, 1024]) ----
    negfg = gpool.tile([32, 1], F32, tag="negfg")
    nc.scalar.mul(negfg[:], fg_all[:], -1.0)
    t_all = tpool.tile([32, 1024], F32, tag="tmp40", name="t_all")
    nc.scalar.activation(t_all[:], prw_all[:], AF.Identity, bias=1.0, scale=negfg[:])
    lt_all = tpool.tile([32, 1024], F32, tag="tmp40", name="lt_all")
    nc.scalar.activation(lt_all[:], t_all[:], AF.Ln)
    psum_phi = ps_big.tile([128, 1024], F32, tag="pbig", name="psum_phi")
    for h in range(2):
        nc.tensor.matmul(psum_phi[0:8, ts(h, 512)], selE4[:, :], lt_all[:, ts(h, 512)],
                         start=True, stop=True)
    phi8 = tpool.tile([8, 1024], F32, tag="tmp40", name="phi8")
    nc.scalar.activation(phi8[:], psum_phi[0:8, :], AF.Exp)

    onem_pu8 = tpool.tile([8, 1024], F32, tag="tmp40", name="onem_pu8")
    nc.scalar.activation(onem_pu8[:], pu8[:], AF.Identity, bias=1.0, scale=-1.0)
    onem_pww8 = tpool.tile([8, 1024], F32, tag="tmp40", name="onem_pww8")
    nc.scalar.activation(onem_pww8[:], pww8[:], AF.Identity, bias=1.0, scale=-1.0)
    mm2 = tpool.tile([8, 1024], F32, tag="tmp40", name="mm2")
    nc.vector.tensor_mul(mm2[:], onem_pu8[:], onem_pww8[:])
    u18 = tpool.tile([8, 1024], F32, tag="tmp40", name="u18")
    nc.scalar.activation(u18[:], mm2[:], AF.Identity, bias=1.0, scale=-1.0)
    usage8 = tpool.tile([8, 1024], F32, tag="tmp40", name="usage8")
    nc.vector.tensor_mul(usage8[:], u18[:], phi8[:])
    nc.sync.dma_start(ou_d[:, :], usage8[:])

    ua8 = gpool.tile([8, 1024], F32, tag="ua8")
    nc.scalar.activation(ua8[:], usage8[:], AF.Identity, bias=epsv[0:8, :], scale=1.0 - EPS)
    lua8 = tpool.tile([8, 1024], F32, tag="tmp40", name="lua8")
    nc.scalar.activation(lua8[:], ua8[:], AF.Ln)
    nonu8 = gpool.tile([8, 1024], F32, tag="nonu8")
    nc.scalar.activation(nonu8[:], ua8[:], AF.Identity, bias=1.0, scale=-1.0)

    # ---- phase 2: column layouts of ua / log(ua) : [128, 8*T] ----
    ucol = gpool.tile([128, 64], F32, tag="ucol")   # [:, t*8+e]
    lcol = gpool.tile([128, 64], F32, tag="lcol")
    for t in range(T):
        p1 = ps_tr.tile([128, 8], F32, tag="tr")
        nc.tensor.transpose(p1[:], ua8[:, ts(t, 128)], ident[0:8, 0:8])
        nc.scalar.copy(ucol[:, ts(t, 8)], p1[:])
        p2 = ps_tr.tile([128, 8], F32, tag="tr")
        nc.tensor.transpose(p2[:], lua8[:, ts(t, 128)], ident[0:8, 0:8])
        nc.scalar.copy(lcol[:, ts(t, 8)], p2[:])

    # sign-trick prep: with CT' = sign(u_i - u_j) and g = log(ua) - mean,
    #   alloc = nonu/sqrt(ua) * exp(0.5*(s'_c + m*r) + Stot/2)
    # where s'_c = sum_j sign_ij*g_j, r = sum_j sign_ij (rank statistic) and
    # the mean-centering keeps the fp32 psum accumulation well conditioned.
    negucol = gpool.tile([128, 64], F32, tag="negucol")
    nc.scalar.mul(negucol[:], ucol[:], -1.0)
    sh8 = gpool.tile([8, 1], F32, tag="sh8")
    nc.vector.reduce_sum(sh8[:], lua8[:], axis=AX.X)
    p_sh = ps_tr.tile([1, 8], F32, tag="tr", name="p_sh")
    nc.tensor.transpose(p_sh[:], sh8[:], ident[0:8, 0:8])
    sh_half = gpool.tile([1, 8], F32, tag="sh_half")
    nc.scalar.mul(sh_half[:], p_sh[:], 0.5)
    mrow = gpool.tile([1, 8], F32, tag="mrow")
    nc.scalar.mul(mrow[:], sh_half[:], 1.0 / 512.0)
    negm8 = gpool.tile([8, 1], F32, tag="negm8")
    nc.scalar.mul(negm8[:], sh8[:], -1.0 / 1024.0)
    luac8 = tpool.tile([8, 1024], F32, tag="tmp40", name="luac8")
    nc.scalar.activation(luac8[:], lua8[:], AF.Identity, bias=negm8[:])
    lcolc = gpool.tile([128, 64], F32, tag="lcolc")
    for t in range(T):
        p5 = ps_tr.tile([128, 8], F32, tag="tr", name="p5")
        nc.tensor.transpose(p5[:], luac8[:, ts(t, 128)], ident[0:8, 0:8])
        nc.scalar.copy(lcolc[:, ts(t, 8)], p5[:])
    # interleave centered log columns with a ones column: lhsT [128, 2]
    lcol2 = gpool.tile([128, 128], F32, tag="lcol2")
    nc.scalar.copy(
        lcol2[:].rearrange("p (c two) -> p c two", two=2)[:, :, 0],
        lcolc[:])
    nc.vector.memset(lcol2[:].rearrange("p (c two) -> p c two", two=2)[:, :, 1:2], 1.0)
    # per-element [2, 1] combine coefficients [1, m_e] for the K=2 matmul
    mcoef = gpool.tile([2, 8], F32, tag="mcoef")
    nc.vector.memset(mcoef[0:1, :], 1.0)
    nc.sync.dma_start(mcoef[1:2, :], mrow[:])

    # ---- phase 3: allocation weights per element ----
    es8 = gpool.tile([8, 1024], F32, tag="es8")
    for e in range(E):
        psum_ub = ps_big.tile([128, 1024], F32, tag="pbig")
        for h in range(2):
            nc.tensor.matmul(psum_ub[:, ts(h, 512)], sel8[:, ts(e, 128)],
                             ua8[:, ts(h, 512)], start=True, stop=True)
        ub_sb = epool.tile([128, 1024], F32, tag="ub_sb")
        nc.scalar.copy(ub_sb[:], psum_ub[:])

        if CMP_IMPL == "act":
            ps_sr = [ps_acc.tile([2, 512], F32, tag="pacc", name=f"ps_sr{h}") for h in range(2)]
        else:
            ps_sr = [ps_acc.tile([2, 512], F32, tag="pacc", name=f"ps_s{h}") for h in range(2)]
        for t in range(T):
            ct = ctpool.tile([128, 1024], F32, tag="CT")
            c = t * 8 + e
            if CMP_IMPL == "act":
                nc.scalar.sign(ct[:], ub_sb[:], bias=negucol[:, c:c + 1])
                for h in range(2):
                    nc.tensor.matmul(ps_sr[h][:, :], lcol2[:, 2 * c:2 * c + 2],
                                     ct[:, ts(h, 512)], start=(t == 0), stop=(t == T - 1))
            else:
                eng = nc.gpsimd if t < GPS_CMP else nc.vector
                eng.tensor_scalar(ct[:], ub_sb[:], ucol[:, c:c + 1],
                                  None, OP.is_gt)
                for h in range(2):
                    nc.tensor.matmul(ps_sr[h][0:1, :], lcol[:, c:c + 1],
                                     ct[:, ts(h, 512)], start=(t == 0), stop=(t == T - 1))
        es_e = e1pool.tile([1, 1024], F32, tag="es_e")
        if CMP_IMPL == "act":
            sc2 = e1pool.tile([2, 1024], F32, tag="sc2")
            for h in range(2):
                nc.scalar.copy(sc2[:, ts(h, 512)], ps_sr[h][:, :])
            for h in range(2):
                ps_c = ps_acc.tile([1, 512], F32, tag="pacc", name=f"ps_c{h}")
                nc.tensor.matmul(ps_c[:, :], mcoef[:, e:e + 1], sc2[:, ts(h, 512)],
                                 start=True, stop=True)
                nc.scalar.activation(es_e[:, ts(h, 512)], ps_c[:, :], AF.Exp,
                                     bias=sh_half[0:1, e:e + 1], scale=0.5)
        else:
            for h in range(2):
                nc.scalar.activation(es_e[:, ts(h, 512)], ps_sr[h][0:1, :], AF.Exp)
        nc.sync.dma_start(es8[e:e + 1, :], es_e[:])

    alloc8 = gpool.tile([8, 1024], F32, tag="alloc8")
    if CMP_IMPL == "act":
        # sign-trick self-term correction: s = (s' + Stot)/2 - log(ua_i)/2,
        # folded in as alloc = (nonu / sqrt(ua)) * exp((s' + Stot)/2)
        squ8 = tpool.tile([8, 1024], F32, tag="tmp40", name="squ8")
        nc.scalar.activation(squ8[:], ua8[:], AF.Sqrt)
        isq8 = tpool.tile([8, 1024], F32, tag="tmp40", name="isq8")
        nc.vector.reciprocal(isq8[:], squ8[:])
        pref8 = tpool.tile([8, 1024], F32, tag="tmp40", name="pref8")
        nc.vector.tensor_mul(pref8[:], nonu8[:], isq8[:])
        nc.vector.tensor_mul(alloc8[:], pref8[:], es8[:])
    else:
        nc.vector.tensor_mul(alloc8[:], nonu8[:], es8[:])

    # ---- phase 4: cosine content weights ----
    # keys: transpose [40, 64] -> [64, 40]
    p_k = ps_tr.tile([64, 40], F32, tag="tr")
    nc.tensor.transpose(p_k[:], K40[:], ident[0:40, 0:40])
    k5t = gpool.tile([64, 40], F32, tag="k5t")
    nc.scalar.copy(k5t[:], p_k[:])
    # key norms and softplus(strengths)
    sqk = gpool.tile([40, 64], F32, tag="sqk")
    nc.scalar.activation(sqk[:], K40[:], AF.Square)
    kn2 = gpool.tile([40, 1], F32, tag="kn2")
    nc.vector.reduce_sum(kn2[:], sqk[:], axis=AX.X)
    kn40 = gpool.tile([40, 1], F32, tag="kn40")
    nc.scalar.activation(kn40[:], kn2[:], AF.Sqrt, bias=epsv[0:40, :])
    e40 = gpool.tile([40, 1], F32, tag="e40")
    nc.scalar.activation(e40[:], st40[:], AF.Exp)
    sp40 = gpool.tile([40, 1], F32, tag="sp40")
    nc.scalar.activation(sp40[:], e40[:], AF.Ln, bias=1.0)

    mn8 = gpool.tile([8, 1024], F32, tag="mn8")
    d40 = gpool.tile([40, 1024], F32, tag="d40")
    for e in range(E):
        mem_e = epool.tile([128, 512], F32, tag="mem_e")
        nc.sync.dma_start(
            mem_e[:].rearrange("p (t w) -> p t w", w=W),
            mem_d[e].rearrange("(t p) w -> p t w", p=128),
        )
        memt = e1pool.tile([64, 1024], F32, tag="memt")
        for t in range(T):
            p3 = ps_tr.tile([64, 128], F32, tag="tr")
            nc.tensor.transpose(p3[:], mem_e[:, ts(t, W)], ident[:, :])
            nc.scalar.copy(memt[:, ts(t, 128)], p3[:])
        sq_e = e1pool.tile([64, 1024], F32, tag="sq_e")
        nc.scalar.activation(sq_e[:], memt[:], AF.Square)
        ps_mn = [ps_acc.tile([1, 512], F32, tag="pacc", name=f"ps_mn{h}") for h in range(2)]
        for h in range(2):
            nc.tensor.matmul(ps_mn[h][:, :], ones64[:, :], sq_e[:, ts(h, 512)],
                             start=True, stop=True)
        mn_e = e1pool.tile([1, 1024], F32, tag="mn_e")
        for h in range(2):
            nc.scalar.activation(mn_e[:, ts(h, 512)], ps_mn[h][:, :], AF.Sqrt, bias=epsv[0:1, :])
        nc.sync.dma_start(mn8[e:e + 1, :], mn_e[:])

        ps_d5 = ps_big.tile([128, 1024], F32, tag="pbig", name="ps_d5")
        for h in range(2):
            nc.tensor.matmul(ps_d5[0:H, ts(h, 512)], k5t[:, ts(e, H)],
                             memt[:, ts(h, 512)], start=True, stop=True)
        d5sb = e1pool.tile([5, 1024], F32, tag="d5sb")
        nc.scalar.copy(d5sb[:], ps_d5[0:H, :])
        nc.sync.dma_start(d40[e * H:(e + 1) * H, :], d5sb[:])

    # norms denominator: mn40[5e+h, n] = mn8[e, n]
    ps_mn40 = ps_big.tile([128, 1024], F32, tag="pbig", name="ps_mn40")
    for h in range(2):
        nc.tensor.matmul(ps_mn40[0:40, ts(h, 512)], selM[:, :], mn8[:, ts(h, 512)],
                         start=True, stop=True)
    nd40 = tpool.tile([40, 1024], F32, tag="tmp40", name="nd40")
    nc.scalar.activation(nd40[:], ps_mn40[0:40, :], AF.Identity, bias=epsv[0:40, :], scale=kn40[:])
    rec40 = tpool.tile([40, 1024], F32, tag="tmp40", name="rec40")
    nc.vector.reciprocal(rec40[:], nd40[:])
    recsp40 = tpool.tile([40, 1024], F32, tag="tmp40", name="recsp40")
    nc.vector.tensor_scalar(recsp40[:], rec40[:], sp40[:], None, OP.mult)
    sharp40 = tpool.tile([40, 1024], F32, tag="tmp40", name="sharp40")
    nc.vector.tensor_mul(sharp40[:], d40[:], recsp40[:])

    # softmax over n (batched over all 40 head-rows)
    mx40 = gpool.tile([40, 1], F32, tag="mx40")
    nc.vector.reduce_max(mx40[:], sharp40[:], axis=AX.X)
    negmx = gpool.tile([40, 1], F32, tag="negmx")
    nc.scalar.mul(negmx[:], mx40[:], -1.0)
    ex40 = tpool.tile([40, 1024], F32, tag="tmp40", name="ex40")
    sumex = gpool.tile([40, 1], F32, tag="sumex")
    nc.scalar.activation(ex40[:], sharp40[:], AF.Exp, bias=negmx[:], accum_out=sumex[:])
    inv40 = gpool.tile([40, 1], F32, tag="inv40")
    nc.vector.reciprocal(inv40[:], sumex[:])
    sm40 = gpool.tile([40, 1024], F32, tag="sm40")
    nc.vector.tensor_scalar(sm40[:], ex40[:], inv40[:], None, OP.mult)
    wc8 = gpool.tile([8, 1024], F32, tag="wc8")
    for e in range(E):
        nc.sync.dma_start(orw_d[e, :, :], sm40[e * H:e * H + R, :])
        nc.sync.dma_start(wc8[e:e + 1, :], sm40[e * H + R:e * H + H, :])

    # ---- phase 5: write weights + precedence (batched [8, 1024]) ----
    onem_ag8 = gpool.tile([8, 1], F32, tag="onem_ag8")
    nc.scalar.activation(onem_ag8[:], ag8[:], AF.Identity, bias=1.0, scale=-1.0)
    c1_8 = gpool.tile([8, 1], F32, tag="c1_8")
    nc.vector.tensor_mul(c1_8[:], wg8[:], ag8[:])
    c2_8 = gpool.tile([8, 1], F32, tag="c2_8")
    nc.vector.tensor_mul(c2_8[:], wg8[:], onem_ag8[:])
    wpart8 = gpool.tile([8, 1024], F32, tag="wpart8")
    nc.vector.tensor_scalar(wpart8[:], wc8[:], c2_8[:], None, OP.mult)
    w8 = gpool.tile([8, 1024], F32, tag="w8")
    wsum8 = gpool.tile([8, 1], F32, tag="wsum8")
    nc.vector.scalar_tensor_tensor(w8[:], alloc8[:], c1_8[:], wpart8[:],
                                   OP.mult, OP.add, accum_out=wsum8[:])
    nc.sync.dma_start(oww_d[:, 0, :], w8[:])
    onem_ws8 = gpool.tile([8, 1], F32, tag="onem_ws8")
    nc.scalar.activation(onem_ws8[:], wsum8[:], AF.Identity, bias=1.0, scale=-1.0)
    prec8 = gpool.tile([8, 1024], F32, tag="prec8")
    nc.vector.scalar_tensor_tensor(prec8[:], pp8[:], onem_ws8[:], w8[:],
                                   OP.mult, OP.add)
    nc.sync.dma_start(op_d[:, 0, :], prec8[:])

    # w columns for the link update: wcol[:, t*8+e] = w8[e, t*128+p]
    wcol = gpool.tile([128, 64], F32, tag="wcol")
    for t in range(T):
        p4 = ps_tr.tile([128, 8], F32, tag="tr")
        nc.tensor.transpose(p4[:], w8[:, ts(t, 128)], ident[0:8, 0:8])
        nc.scalar.copy(wcol[:, ts(t, 8)], p4[:])
    acol = gpool.tile([128, 64], F32, tag="acol")
    nc.scalar.activation(acol[:], wcol[:], AF.Identity, bias=1.0, scale=-1.0)

    # ---- phase 6: link update (the bulk) ----
    for e in range(E):
        psum_wb = ps_big.tile([128, 1024], F32, tag="pbig")
        for h in range(2):
            nc.tensor.matmul(psum_wb[:, ts(h, 512)], sel8[:, ts(e, 128)],
                             w8[:, ts(h, 512)], start=True, stop=True)
        wn_e = epool.tile([128, 1024], F32, tag="wn_e")
        nc.scalar.mul(wn_e[:], psum_wb[:], -1.0)

        psum_pb = ps_big.tile([128, 1024], F32, tag="pbig")
        for h in range(2):
            nc.tensor.matmul(psum_pb[:, ts(h, 512)], sel8[:, ts(e, 128)],
                             pp8[:, ts(h, 512)], start=True, stop=True)
        pb_e = epool.tile([128, 1024], F32, tag="pb_e")
        nc.scalar.copy(pb_e[:], psum_pb[:])

        for t in range(T):
            lt = lpool.tile([128, 1024], F32, tag="L")
            nc.sync.dma_start(lt[:], pl_d[e, 0, ts(t, 128), :])
            ot = opool.tile([128, 1024], F32, tag="O")
            a_sc = acol[:, t * 8 + e: t * 8 + e + 1]
            w_sc = wcol[:, t * 8 + e: t * 8 + e + 1]
            if t < GPS_STT:
                cgen = cgpool.tile([128, 1024], F32, tag="cgen", name="cgen")
                nc.scalar.activation(cgen[:], wn_e[:], AF.Identity, bias=a_sc)
                nc.gpsimd.tensor_mul(ot[:], cgen[:], lt[:])
            else:
                nc.vector.scalar_tensor_tensor(ot[:], wn_e[:], a_sc, lt[:],
                                               OP.add, OP.mult)
            nc.vector.scalar_tensor_tensor(ot[:], pb_e[:], w_sc, ot[:],
                                           OP.mult, OP.add)
            diag_eng = nc.gpsimd if DIAG_ENG == "pool" else nc.vector
            diag_eng.tensor_mul(ot[:, ts(t, 128)], ot[:, ts(t, 128)], dmask[:])
            nc.sync.dma_start(ol_d[e, 0, ts(t, 128), :], ot[:])

    ctx.close()


def _build():
    global _NC
    if _NC is not None:
        return _NC
    nc = bacc.Bacc("TRN2", target_bir_lowering=False, debug=False, num_devices=NCORES)
    mem_d = nc.dram_tensor("memory", [E, N, W], F32, kind="ExternalInput")
    rk_d = nc.dram_tensor("read_keys", [E, R, W], F32, kind="ExternalInput")
    rs_d = nc.dram_tensor("read_strengths", [E, R], F32, kind="ExternalInput")
    wk_d = nc.dram_tensor("write_keys", [E, NW, W], F32, kind="ExternalInput")
    ws_d = nc.dram_tensor("write_strengths", [E, NW], F32, kind="ExternalInput")
    fg_d = nc.dram_tensor("free_gate", [E, R], F32, kind="ExternalInput")
    ag_d = nc.dram_tensor("alloc_gate", [E, NW], F32, kind="ExternalInput")
    wg_d = nc.dram_tensor("write_gate", [E, NW], F32, kind="ExternalInput")
    prw_d = nc.dram_tensor("prev_read_weights", [E, R, N], F32, kind="ExternalInput")
    pww_d = nc.dram_tensor("prev_write_weights", [E, NW, N], F32, kind="ExternalInput")
    pu_d = nc.dram_tensor("prev_usage", [E, N], F32, kind="ExternalInput")
    pl_d = nc.dram_tensor("prev_link", [E, NW, N, N], F32, kind="ExternalInput")
    pp_d = nc.dram_tensor("prev_precedence", [E, NW, N], F32, kind="ExternalInput")
    ident_d = nc.dram_tensor("c_ident", [128, 128], F32, kind="ExternalInput")
    dmask_d = nc.dram_tensor("c_dmask", [128, 128], F32, kind="ExternalInput")
    sel8_d = nc.dram_tensor("c_sel8", [8, 1024], F32, kind="ExternalInput")
    selE4_d = nc.dram_tensor("c_selE4", [32, 8], F32, kind="ExternalInput")
    selM_d = nc.dram_tensor("c_selM", [8, 40], F32, kind="ExternalInput")
    orw_d = nc.dram_tensor("o_read_weights", [E, R, N], F32, kind="ExternalOutput")
    oww_d = nc.dram_tensor("o_write_weights", [E, NW, N], F32, kind="ExternalOutput")
    ou_d = nc.dram_tensor("o_usage", [E, N], F32, kind="ExternalOutput")
    ol_d = nc.dram_tensor("o_link", [E, NW, N, N], F32, kind="ExternalOutput")
    op_d = nc.dram_tensor("o_precedence", [E, NW, N], F32, kind="ExternalOutput")
    io = (mem_d, rk_d, rs_d, wk_d, ws_d, fg_d, ag_d, wg_d, prw_d, pww_d, pu_d,
          pl_d, pp_d, ident_d, dmask_d, sel8_d, selE4_d, selM_d,
          orw_d, oww_d, ou_d, ol_d, op_d)
    with tile.TileContext(nc) as tc:
        _emit(nc, tc, io)
    nc.compile()
    _NC = nc
    return nc


def _consts():
    eye = np.eye(128, dtype=np.float32)
    return {
        "c_ident": eye,
        "c_dmask": (1.0 - eye).astype(np.float32),
        "c_sel8": np.repeat(np.eye(8, dtype=np.float32), 128, axis=1),
        "c_selE4": np.repeat(np.eye(8, dtype=np.float32), 4, axis=0),
        "c_selM": np.repeat(np.eye(8, dtype=np.float32), 5, axis=1),
    }


def kernel(memory, read_keys, read_strengths, write_keys, write_strengths,
           free_gate, alloc_gate, write_gate, prev_read_weights,
           prev_write_weights, prev_usage, prev_link, prev_precedence):
    global LAST_RESULTS
    nc = _build()
    full = {
        "memory": memory, "read_keys": read_keys,
        "read_strengths": read_strengths, "write_keys": write_keys,
        "write_strengths": write_strengths, "free_gate": free_gate,
        "alloc_gate": alloc_gate, "write_gate": write_gate,
        "prev_read_weights": prev_read_weights,
        "prev_write_weights": prev_write_weights, "prev_usage": prev_usage,
        "prev_link": prev_link, "prev_precedence": prev_precedence,
    }
    consts = _consts()
    in_maps = []
    for c in range(NCORES):
        m = {k: np.ascontiguousarray(np.asarray(v)[c * E:(c + 1) * E],
                                     dtype=np.float32)
             for k, v in full.items()}
        m.update(consts)
        in_maps.append(m)
    res = run_bass_kernel_spmd(nc, in_maps, core_ids=list(range(NCORES)),
                               trace=TRACE, **TRACE_KW)
    LAST_RESULTS = res
    outs = res.results
    read_weights = np.concatenate([outs[c]["o_read_weights"] for c in range(NCORES)], 0)
    write_weights = np.concatenate([outs[c]["o_write_weights"] for c in range(NCORES)], 0)
    usage = np.concatenate([outs[c]["o_usage"] for c in range(NCORES)], 0)
    link = np.concatenate([outs[c]["o_link"] for c in range(NCORES)], 0)
    precedence = np.concatenate([outs[c]["o_precedence"] for c in range(NCORES)], 0)
    return (read_weights, write_weights, usage, link, precedence)


# revision 20
# speedup vs baseline: 1.7545x; 1.2380x over previous
"""Trainium2 Bass kernel for one DNC memory-addressing timestep.

Contract: kernel(**inputs) takes the FULL (unsharded) numpy inputs of
reference.setup_inputs() and returns the full outputs
(read_weights, write_weights, usage, link, precedence) as numpy float32.

Sharding: pure data parallel over batch dim 0 across 8 NeuronCores
(8 batch elements per core, no cross-core communication).

Algorithm notes (per batch element, N=1024 slots, W=64 word, R=4 reads):
  - usage / precedence / write weights: exact elementwise math, batched
    across the 8 per-core elements on partitions 0..7 ("row layout"
    [8, 1024]) so every vector op costs one instruction per core.
  - allocation: the reference sorts usage; here we use the equivalent
    closed form alloc[i] = nonusage[i] * prod_{j: u_j < u_i} u_j
                        = nonusage[i] * exp(sum_j [u_j < u_i] * log u_j)
    computed with comparison tiles (DVE is_gt) contracted against
    log(u) columns on the TensorEngine.  (fp32 ties have ~0 probability
    and were validated against the reference to 3e-8 absmax.)
  - cosine content addressing: dot products via PE after on-chip
    transposes of memory; softmax batched over all 8*5 head-rows.
  - link update (the memory-bound bulk: 8MB/elt of HBM traffic) is two
    fused scalar_tensor_tensor ops per [128, 1024] tile:
        out = (a_i - w_j) * L         (a = 1 - w, per-partition scalar)
        out = (p_j * w_i) + out
    plus a [128,128] diagonal mask multiply, split DVE/GPSIMD.
"""

import os
import sys
from contextlib import ExitStack

for _p in ("/opt/trn_rl_repo", "/root/.axon_site/_ro/trn_rl_repo"):
    if os.path.isdir(_p) and _p not in sys.path:
        sys.path.append(_p)

import numpy as np

import concourse.bass as bass
import concourse.bacc as bacc
import concourse.mybir as mybir
import concourse.tile as tile
from concourse.bass_utils import run_bass_kernel_spmd

F32 = mybir.dt.float32
F32R = mybir.dt.float32r
BF16 = mybir.dt.bfloat16
AF = mybir.ActivationFunctionType
OP = mybir.AluOpType
AX = mybir.AxisListType
ts = bass.ts

EPS = 1e-5
B, N, W, R, NW = 64, 1024, 64, 4, 1
NCORES = 8
E = B // NCORES          # batch elements per core = 8
T = N // 128             # 128-row chunks per link matrix = 8
H = R + NW               # stacked heads (4 read + 1 write) = 5

# How many of the 8 link-update "first op" tiles per element run on
# GPSIMD instead of DVE (load balancing knob).
GPS_STT = int(os.environ.get("K_GPS_STT", "0"))
# Engine for the diagonal mask multiply: "pool" or "dve".
DIAG_ENG = os.environ.get("K_DIAG_ENG", "pool")

TRACE = False
TRACE_KW = {}
LAST_RESULTS = None

_NC = None


def _emit(nc, tc, io):
    (mem_d, rk_d, rs_d, wk_d, ws_d, fg_d, ag_d, wg_d, prw_d, pww_d, pu_d,
     pl_d, pp_d, ident_d, dmask_d, sel8_d, selE4_d, selM_d,
     orw_d, oww_d, ou_d, ol_d, op_d) = io

    ctx = ExitStack()
    cpool = ctx.enter_context(tc.tile_pool(name="consts", bufs=1))
    gpool = ctx.enter_context(tc.tile_pool(name="glob", bufs=1))
    tpool = ctx.enter_context(tc.tile_pool(name="tmp", bufs=5))
    epool = ctx.enter_context(tc.tile_pool(name="pere", bufs=2))
    e1pool = ctx.enter_context(tc.tile_pool(name="pere1", bufs=1))
    ctpool = ctx.enter_context(tc.tile_pool(name="ct", bufs=3))
    lpool = ctx.enter_context(tc.tile_pool(name="lin", bufs=9))
    cgpool = ctx.enter_context(tc.tile_pool(name="cg", bufs=2))
    opool = ctx.enter_context(tc.tile_pool(name="lout", bufs=3))
    ps_big = ctx.enter_context(tc.tile_pool(name="ps_big", bufs=1, space="PSUM"))
    ps_acc = ctx.enter_context(tc.tile_pool(name="ps_acc", bufs=4, space="PSUM"))
    ps_tr = ctx.enter_context(tc.tile_pool(name="ps_tr", bufs=2, space="PSUM"))

    # ---- constants ----
    ident = cpool.tile([128, 128], F32, tag="ident")
    dmask = cpool.tile([128, 128], F32, tag="dmask")
    sel8 = cpool.tile([8, 1024], F32, tag="sel8")
    selE4 = cpool.tile([32, 8], F32, tag="selE4")
    selM = cpool.tile([8, 40], F32, tag="selM")
    epsv = cpool.tile([128, 1], F32, tag="epsv")
    nc.sync.dma_start(ident[:], ident_d[:])
    nc.sync.dma_start(dmask[:], dmask_d[:])
    nc.sync.dma_start(sel8[:], sel8_d[:])
    nc.sync.dma_start(selE4[:], selE4_d[:])
    nc.sync.dma_start(selM[:], selM_d[:])
    nc.vector.memset(epsv[:], EPS)

    # ---- small input loads ----
    prw_all = gpool.tile([32, 1024], F32, tag="prw")       # rows 4e+r
    fg_all = gpool.tile([32, 1], F32, tag="fg")
    pu8 = gpool.tile([8, 1024], F32, tag="pu8")
    pww8 = gpool.tile([8, 1024], F32, tag="pww8")
    pp8 = gpool.tile([8, 1024], F32, tag="pp8")
    ag8 = gpool.tile([8, 1], F32, tag="ag8")
    wg8 = gpool.tile([8, 1], F32, tag="wg8")
    K40 = gpool.tile([40, 64], F32, tag="K40")             # rows 5e+h
    st40 = gpool.tile([40, 1], F32, tag="st40")
    nc.sync.dma_start(prw_all[:], prw_d[:, :, :].rearrange("e r n -> (e r) n"))
    nc.sync.dma_start(fg_all[:, 0], fg_d[:, :].rearrange("e r -> (e r)"))
    nc.sync.dma_start(pu8[:], pu_d[:, :])
    nc.sync.dma_start(pww8[:], pww_d[:, 0, :])
    nc.sync.dma_start(pp8[:], pp_d[:, 0, :])
    nc.sync.dma_start(ag8[:], ag_d[:, :])
    nc.sync.dma_start(wg8[:], wg_d[:, :])
    for e in range(E):
        nc.sync.dma_start(K40[e * H:e * H + R, :], rk_d[e, :, :])
        nc.sync.dma_start(K40[e * H + R:e * H + H, :], wk_d[e, :, :])
        nc.sync.dma_start(st40[e * H:e * H + R, 0], rs_d[e, :])
        nc.sync.dma_start(st40[e * H + R:e * H + H, 0], ws_d[e, :])

    # ---- phase 1: usage (batched rows [8, 1024]) ----
    negfg = gpool.tile([32, 1], F32, tag="negfg")
    nc.scalar.mul(negfg[:], fg_all[:], -1.0)
    t_all = tpool.tile([32, 1024], F32, tag="tmp40", name="t_all")
    nc.scalar.activation(t_all[:], prw_all[:], AF.Identity, bias=1.0, scale=negfg[:])
    lt_all = tpool.tile([32, 1024], F32, tag="tmp40", name="lt_all")
    nc.scalar.activation(lt_all[:], t_all[:], AF.Ln)
    psum_phi = ps_big.tile([128, 1024], F32, tag="pbig", name="psum_phi")
    for h in range(2):
        nc.tensor.matmul(psum_phi[0:8, ts(h, 512)], selE4[:, :], lt_all[:, ts(h, 512)],
                         start=True, stop=True)
    phi8 = tpool.tile([8, 1024], F32, tag="tmp40", name="phi8")
    nc.scalar.activation(phi8[:], psum_phi[0:8, :], AF.Exp)

    onem_pu8 = tpool.tile([8, 1024], F32, tag="tmp40", name="onem_pu8")
    nc.scalar.activation(onem_pu8[:], pu8[:], AF.Identity, bias=1.0, scale=-1.0)
    onem_pww8 = tpool.tile([8, 1024], F32, tag="tmp40", name="onem_pww8")
    nc.scalar.activation(onem_pww8[:], pww8[:], AF.Identity, bias=1.0, scale=-1.0)
    mm2 = tpool.tile([8, 1024], F32, tag="tmp40", name="mm2")
    nc.vector.tensor_mul(mm2[:], onem_pu8[:], onem_pww8[:])
    u18 = tpool.tile([8, 1024], F32, tag="tmp40", name="u18")
    nc.scalar.activation(u18[:], mm2[:], AF.Identity, bias=1.0, scale=-1.0)
    usage8 = tpool.tile([8, 1024], F32, tag="tmp40", name="usage8")
    nc.vector.tensor_mul(usage8[:], u18[:], phi8[:])
    nc.scalar.dma_start(ou_d[:, :], usage8[:])

    ua8 = gpool.tile([8, 1024], F32, tag="ua8")
    nc.scalar.activation(ua8[:], usage8[:], AF.Identity, bias=epsv[0:8, :], scale=1.0 - EPS)
    lua8 = tpool.tile([8, 1024], F32, tag="tmp40", name="lua8")
    nc.scalar.activation(lua8[:], ua8[:], AF.Ln)
    nonu8 = gpool.tile([8, 1024], F32, tag="nonu8")
    nc.scalar.activation(nonu8[:], ua8[:], AF.Identity, bias=1.0, scale=-1.0)

    # ---- phase 2: column layouts of ua / log(ua) : [128, 8*T] ----
    ucol = gpool.tile([128, 64], F32, tag="ucol")   # [:, t*8+e]
    lcol = gpool.tile([128, 64], F32, tag="lcol")
    for t in range(T):
        p1 = ps_tr.tile([128, 8], F32, tag="tr")
        nc.tensor.transpose(p1[:], ua8[:, ts(t, 128)], ident[0:8, 0:8])
        nc.scalar.copy(ucol[:, ts(t, 8)], p1[:])
        p2 = ps_tr.tile([128, 8], F32, tag="tr")
        nc.tensor.transpose(p2[:], lua8[:, ts(t, 128)], ident[0:8, 0:8])
        nc.scalar.copy(lcol[:, ts(t, 8)], p2[:])

    # The rank-product matmul runs in bf16 for PE speed: the 0/1 compare
    # matrix is exact in bf16, and log(ua) is split into three bf16
    # components (hi+mid+lo reproduces fp32 to ~2^-26 relative).
    l3 = gpool.tile([128, 192], BF16, tag="l3")
    l3v = l3[:].rearrange("p (c k) -> p c k", k=3)
    hi_f = gpool.tile([128, 64], F32, tag="hi_f")
    r1 = gpool.tile([128, 64], F32, tag="r1")
    nc.scalar.copy(l3v[:, :, 0], lcol[:])
    nc.scalar.copy(hi_f[:], l3v[:, :, 0])
    nc.vector.tensor_sub(r1[:], lcol[:], hi_f[:])
    nc.scalar.copy(l3v[:, :, 1], r1[:])
    nc.scalar.copy(hi_f[:], l3v[:, :, 1])
    nc.vector.tensor_sub(r1[:], r1[:], hi_f[:])
    nc.scalar.copy(l3v[:, :, 2], r1[:])
    ones3 = cpool.tile([3, 1], F32, tag="ones3")
    nc.vector.memset(ones3[:], 1.0)

    # ---- phase 3: allocation weights per element ----
    es8 = gpool.tile([8, 1024], F32, tag="es8")
    for e in range(E):
        psum_ub = ps_big.tile([128, 1024], F32, tag="pbig")
        for h in range(2):
            nc.tensor.matmul(psum_ub[:, ts(h, 512)], sel8[:, ts(e, 128)],
                             ua8[:, ts(h, 512)], start=True, stop=True)
        ub_sb = epool.tile([128, 1024], F32, tag="ub_sb")
        nc.scalar.copy(ub_sb[:], psum_ub[:])

        ps_sr = [ps_acc.tile([3, 512], F32, tag="pacc", name=f"ps_sr{h}") for h in range(2)]
        for t in range(T):
            ct = ctpool.tile([128, 1024], BF16, tag="CT")
            c = t * 8 + e
            nc.vector.tensor_scalar(ct[:], ub_sb[:], ucol[:, c:c + 1],
                                    None, OP.is_gt)
            for h in range(2):
                nc.tensor.matmul(ps_sr[h][:, :], l3[:, 3 * c:3 * c + 3],
                                 ct[:, ts(h, 512)], start=(t == 0), stop=(t == T - 1))
        es_e = e1pool.tile([1, 1024], F32, tag="es_e")
        sc3 = e1pool.tile([3, 1024], F32, tag="sc3")
        for h in range(2):
            nc.scalar.copy(sc3[:, ts(h, 512)], ps_sr[h][:, :])
        for h in range(2):
            ps_c = ps_acc.tile([1, 512], F32, tag="pacc", name=f"ps_c{h}")
            nc.tensor.matmul(ps_c[:, :], ones3[:, :], sc3[:, ts(h, 512)],
                             start=True, stop=True)
            nc.scalar.activation(es_e[:, ts(h, 512)], ps_c[:, :], AF.Exp)
        nc.sync.dma_start(es8[e:e + 1, :], es_e[:])

    alloc8 = gpool.tile([8, 1024], F32, tag="alloc8")
    nc.vector.tensor_mul(alloc8[:], nonu8[:], es8[:])

    # ---- phase 4: cosine content weights ----
    # keys: transpose [40, 64] -> [64, 40]
    p_k = ps_tr.tile([64, 40], F32, tag="tr")
    nc.tensor.transpose(p_k[:], K40[:], ident[0:40, 0:40])
    k5t = gpool.tile([64, 40], F32, tag="k5t")
    nc.scalar.copy(k5t[:], p_k[:])
    # key norms and softplus(strengths)
    sqk = gpool.tile([40, 64], F32, tag="sqk")
    nc.scalar.activation(sqk[:], K40[:], AF.Square)
    kn2 = gpool.tile([40, 1], F32, tag="kn2")
    nc.vector.reduce_sum(kn2[:], sqk[:], axis=AX.X)
    kn40 = gpool.tile([40, 1], F32, tag="kn40")
    nc.scalar.activation(kn40[:], kn2[:], AF.Sqrt, bias=epsv[0:40, :])
    e40 = gpool.tile([40, 1], F32, tag="e40")
    nc.scalar.activation(e40[:], st40[:], AF.Exp)
    sp40 = gpool.tile([40, 1], F32, tag="sp40")
    nc.scalar.activation(sp40[:], e40[:], AF.Ln, bias=1.0)

    # memory norms in column layout via ACT square+accumulate (no PE, no DVE)
    mncol = gpool.tile([128, 64], F32, tag="mncol")   # [:, t*8+e] = ||mem row||^2
    d40 = gpool.tile([40, 1024], F32, tag="d40")
    for e in range(E):
        mem_e = epool.tile([128, 512], F32, tag="mem_e")
        nc.sync.dma_start(
            mem_e[:].rearrange("p (t w) -> p t w", w=W),
            mem_d[e].rearrange("(t p) w -> p t w", p=128),
        )
        memt = e1pool.tile([64, 1024], F32, tag="memt")
        sqd = epool.tile([128, 64], F32, tag="sqd")
        for t in range(T):
            p3 = ps_tr.tile([64, 128], F32, tag="tr", name="p3")
            nc.tensor.transpose(p3[:], mem_e[:, ts(t, W)], ident[:, :])
            nc.scalar.copy(memt[:, ts(t, 128)], p3[:])
            nc.scalar.activation(sqd[:], mem_e[:, ts(t, W)], AF.Square,
                                 accum_out=mncol[:, t * 8 + e:t * 8 + e + 1])

        ps_d5 = ps_big.tile([128, 1024], F32, tag="pbig", name="ps_d5")
        for h in range(2):
            nc.tensor.matmul(ps_d5[0:H, ts(h, 512)], k5t[:, ts(e, H)],
                             memt[:, ts(h, 512)], start=True, stop=True)
        d5sb = e1pool.tile([5, 1024], F32, tag="d5sb")
        nc.scalar.copy(d5sb[:], ps_d5[0:H, :])
        nc.scalar.dma_start(d40[e * H:(e + 1) * H, :], d5sb[:])

    # 1 / sqrt(mn2 + EPS) in columns (cheap reciprocal), back to rows via PE
    isc = gpool.tile([128, 64], F32, tag="isc")
    nc.scalar.activation(isc[:], mncol[:], AF.Sqrt, bias=epsv[:, :])
    nc.vector.reciprocal(isc[:], isc[:])
    imn8 = gpool.tile([8, 1024], F32, tag="imn8")
    for t in range(T):
        p6 = ps_tr.tile([8, 128], F32, tag="tr", name="p6")
        nc.tensor.transpose(p6[:], isc[:, ts(t, 8)], ident[:, :])
        nc.scalar.copy(imn8[:, ts(t, 128)], p6[:])

    # activations scale: (1/kn * softplus(strength)) per head-row; the +EPS in
    # the reference denominator is dropped (rel err <= EPS/(kn*mn) ~ 2e-7)
    ikn40 = gpool.tile([40, 1], F32, tag="ikn40")
    nc.vector.reciprocal(ikn40[:], kn40[:])
    iknsp40 = gpool.tile([40, 1], F32, tag="iknsp40")
    nc.vector.tensor_mul(iknsp40[:], ikn40[:], sp40[:])
    ps_imn40 = ps_big.tile([128, 1024], F32, tag="pbig", name="ps_imn40")
    for h in range(2):
        nc.tensor.matmul(ps_imn40[0:40, ts(h, 512)], selM[:, :], imn8[:, ts(h, 512)],
                         start=True, stop=True)
    z40 = tpool.tile([40, 1024], F32, tag="tmp40", name="z40")
    nc.scalar.mul(z40[:], ps_imn40[0:40, :], iknsp40[:])
    sharp40 = tpool.tile([40, 1024], F32, tag="tmp40", name="sharp40")
    nc.vector.tensor_mul(sharp40[:], d40[:], z40[:])

    # softmax over n (batched over all 40 head-rows)
    mx40 = gpool.tile([40, 1], F32, tag="mx40")
    nc.vector.reduce_max(mx40[:], sharp40[:], axis=AX.X)
    negmx = gpool.tile([40, 1], F32, tag="negmx")
    nc.scalar.mul(negmx[:], mx40[:], -1.0)
    ex40 = tpool.tile([40, 1024], F32, tag="tmp40", name="ex40")
    sumex = gpool.tile([40, 1], F32, tag="sumex")
    nc.scalar.activation(ex40[:], sharp40[:], AF.Exp, bias=negmx[:], accum_out=sumex[:])
    inv40 = gpool.tile([40, 1], F32, tag="inv40")
    nc.vector.reciprocal(inv40[:], sumex[:])
    sm40 = gpool.tile([40, 1024], F32, tag="sm40")
    nc.scalar.mul(sm40[:], ex40[:], inv40[:])
    wc8 = gpool.tile([8, 1024], F32, tag="wc8")
    for e in range(E):
        nc.scalar.dma_start(orw_d[e, :, :], sm40[e * H:e * H + R, :])
        nc.sync.dma_start(wc8[e:e + 1, :], sm40[e * H + R:e * H + H, :])

    # ---- phase 5: write weights + precedence (batched [8, 1024]) ----
    onem_ag8 = gpool.tile([8, 1], F32, tag="onem_ag8")
    nc.scalar.activation(onem_ag8[:], ag8[:], AF.Identity, bias=1.0, scale=-1.0)
    c1_8 = gpool.tile([8, 1], F32, tag="c1_8")
    nc.vector.tensor_mul(c1_8[:], wg8[:], ag8[:])
    c2_8 = gpool.tile([8, 1], F32, tag="c2_8")
    nc.vector.tensor_mul(c2_8[:], wg8[:], onem_ag8[:])
    wpart8 = gpool.tile([8, 1024], F32, tag="wpart8")
    nc.scalar.mul(wpart8[:], wc8[:], c2_8[:])
    w8 = gpool.tile([8, 1024], F32, tag="w8")
    wsum8 = gpool.tile([8, 1], F32, tag="wsum8")
    nc.vector.scalar_tensor_tensor(w8[:], alloc8[:], c1_8[:], wpart8[:],
                                   OP.mult, OP.add, accum_out=wsum8[:])
    nc.scalar.dma_start(oww_d[:, 0, :], w8[:])
    onem_ws8 = gpool.tile([8, 1], F32, tag="onem_ws8")
    nc.scalar.activation(onem_ws8[:], wsum8[:], AF.Identity, bias=1.0, scale=-1.0)
    prec8 = gpool.tile([8, 1024], F32, tag="prec8")
    nc.vector.scalar_tensor_tensor(prec8[:], pp8[:], onem_ws8[:], w8[:],
                                   OP.mult, OP.add)
    nc.scalar.dma_start(op_d[:, 0, :], prec8[:])

    # fp32r copies for the link broadcast matmuls (values tolerate tf32
    # truncation there; rel err ~2^-13 on the broadcasted operand)
    sel8r = gpool.tile([8, 1024], F32R, tag="sel8r")
    nc.scalar.copy(sel8r[:], sel8[:])
    w8r = gpool.tile([8, 1024], F32R, tag="w8r")
    nc.scalar.copy(w8r[:], w8[:])
    pp8r = gpool.tile([8, 1024], F32R, tag="pp8r")
    nc.scalar.copy(pp8r[:], pp8[:])

    # w columns for the link update: wcol[:, t*8+e] = w8[e, t*128+p]
    wcol = gpool.tile([128, 64], F32, tag="wcol")
    for t in range(T):
        p4 = ps_tr.tile([128, 8], F32, tag="tr")
        nc.tensor.transpose(p4[:], w8[:, ts(t, 128)], ident[0:8, 0:8])
        nc.scalar.copy(wcol[:, ts(t, 8)], p4[:])
    acol = gpool.tile([128, 64], F32, tag="acol")
    nc.scalar.activation(acol[:], wcol[:], AF.Identity, bias=1.0, scale=-1.0)

    # ---- phase 6: link update (the bulk) ----
    for e in range(E):
        psum_wb = ps_big.tile([128, 1024], F32, tag="pbig")
        for h in range(2):
            nc.tensor.matmul(psum_wb[:, ts(h, 512)], sel8r[:, ts(e, 128)],
                             w8r[:, ts(h, 512)], start=True, stop=True)
        wn_e = epool.tile([128, 1024], F32, tag="wn_e")
        nc.scalar.mul(wn_e[:], psum_wb[:], -1.0)

        psum_pb = ps_big.tile([128, 1024], F32, tag="pbig")
        for h in range(2):
            nc.tensor.matmul(psum_pb[:, ts(h, 512)], sel8r[:, ts(e, 128)],
                             pp8r[:, ts(h, 512)], start=True, stop=True)
        pb_e = epool.tile([128, 1024], F32, tag="pb_e")
        nc.scalar.copy(pb_e[:], psum_pb[:])

        for t in range(T):
            lt = lpool.tile([128, 1024], F32, tag="L")
            nc.sync.dma_start(lt[:], pl_d[e, 0, ts(t, 128), :])
            ot = opool.tile([128, 1024], F32, tag="O")
            a_sc = acol[:, t * 8 + e: t * 8 + e + 1]
            w_sc = wcol[:, t * 8 + e: t * 8 + e + 1]
            if t < GPS_STT:
                cgen = cgpool.tile([128, 1024], F32, tag="cgen", name="cgen")
                nc.scalar.activation(cgen[:], wn_e[:], AF.Identity, bias=a_sc)
                nc.gpsimd.tensor_mul(ot[:], cgen[:], lt[:])
            else:
                nc.vector.scalar_tensor_tensor(ot[:], wn_e[:], a_sc, lt[:],
                                               OP.add, OP.mult)
            nc.vector.scalar_tensor_tensor(ot[:], pb_e[:], w_sc, ot[:],
                                           OP.mult, OP.add)
            diag_eng = nc.gpsimd if DIAG_ENG == "pool" else nc.vector
            diag_eng.tensor_mul(ot[:, ts(t, 128)], ot[:, ts(t, 128)], dmask[:])
            nc.scalar.dma_start(ol_d[e, 0, ts(t, 128), :], ot[:])

    ctx.close()


def _build():
    global _NC
    if _NC is not None:
        return _NC
    nc = bacc.Bacc("TRN2", target_bir_lowering=False, debug=False, num_devices=NCORES)
    mem_d = nc.dram_tensor("memory", [E, N, W], F32, kind="ExternalInput")
    rk_d = nc.dram_tensor("read_keys", [E, R, W], F32, kind="ExternalInput")
    rs_d = nc.dram_tensor("read_strengths", [E, R], F32, kind="ExternalInput")
    wk_d = nc.dram_tensor("write_keys", [E, NW, W], F32, kind="ExternalInput")
    ws_d = nc.dram_tensor("write_strengths", [E, NW], F32, kind="ExternalInput")
    fg_d = nc.dram_tensor("free_gate", [E, R], F32, kind="ExternalInput")
    ag_d = nc.dram_tensor("alloc_gate", [E, NW], F32, kind="ExternalInput")
    wg_d = nc.dram_tensor("write_gate", [E, NW], F32, kind="ExternalInput")
    prw_d = nc.dram_tensor("prev_read_weights", [E, R, N], F32, kind="ExternalInput")
    pww_d = nc.dram_tensor("prev_write_weights", [E, NW, N], F32, kind="ExternalInput")
    pu_d = nc.dram_tensor("prev_usage", [E, N], F32, kind="ExternalInput")
    pl_d = nc.dram_tensor("prev_link", [E, NW, N, N], F32, kind="ExternalInput")
    pp_d = nc.dram_tensor("prev_precedence", [E, NW, N], F32, kind="ExternalInput")
    ident_d = nc.dram_tensor("c_ident", [128, 128], F32, kind="ExternalInput")
    dmask_d = nc.dram_tensor("c_dmask", [128, 128], F32, kind="ExternalInput")
    sel8_d = nc.dram_tensor("c_sel8", [8, 1024], F32, kind="ExternalInput")
    selE4_d = nc.dram_tensor("c_selE4", [32, 8], F32, kind="ExternalInput")
    selM_d = nc.dram_tensor("c_selM", [8, 40], F32, kind="ExternalInput")
    orw_d = nc.dram_tensor("o_read_weights", [E, R, N], F32, kind="ExternalOutput")
    oww_d = nc.dram_tensor("o_write_weights", [E, NW, N], F32, kind="ExternalOutput")
    ou_d = nc.dram_tensor("o_usage", [E, N], F32, kind="ExternalOutput")
    ol_d = nc.dram_tensor("o_link", [E, NW, N, N], F32, kind="ExternalOutput")
    op_d = nc.dram_tensor("o_precedence", [E, NW, N], F32, kind="ExternalOutput")
    io = (mem_d, rk_d, rs_d, wk_d, ws_d, fg_d, ag_d, wg_d, prw_d, pww_d, pu_d,
          pl_d, pp_d, ident_d, dmask_d, sel8_d, selE4_d, selM_d,
          orw_d, oww_d, ou_d, ol_d, op_d)
    with tile.TileContext(nc) as tc:
        _emit(nc, tc, io)
    nc.compile()
    _NC = nc
    return nc


def _consts():
    eye = np.eye(128, dtype=np.float32)
    return {
        "c_ident": eye,
        "c_dmask": (1.0 - eye).astype(np.float32),
        "c_sel8": np.repeat(np.eye(8, dtype=np.float32), 128, axis=1),
        "c_selE4": np.repeat(np.eye(8, dtype=np.float32), 4, axis=0),
        "c_selM": np.repeat(np.eye(8, dtype=np.float32), 5, axis=1),
    }


def kernel(memory, read_keys, read_strengths, write_keys, write_strengths,
           free_gate, alloc_gate, write_gate, prev_read_weights,
           prev_write_weights, prev_usage, prev_link, prev_precedence):
    global LAST_RESULTS
    nc = _build()
    full = {
        "memory": memory, "read_keys": read_keys,
        "read_strengths": read_strengths, "write_keys": write_keys,
        "write_strengths": write_strengths, "free_gate": free_gate,
        "alloc_gate": alloc_gate, "write_gate": write_gate,
        "prev_read_weights": prev_read_weights,
        "prev_write_weights": prev_write_weights, "prev_usage": prev_usage,
        "prev_link": prev_link, "prev_precedence": prev_precedence,
    }
    consts = _consts()
    in_maps = []
    for c in range(NCORES):
        m = {k: np.ascontiguousarray(np.asarray(v)[c * E:(c + 1) * E],
                                     dtype=np.float32)
             for k, v in full.items()}
        m.update(consts)
        in_maps.append(m)
    res = run_bass_kernel_spmd(nc, in_maps, core_ids=list(range(NCORES)),
                               trace=TRACE, **TRACE_KW)
    LAST_RESULTS = res
    outs = res.results
    read_weights = np.concatenate([outs[c]["o_read_weights"] for c in range(NCORES)], 0)
    write_weights = np.concatenate([outs[c]["o_write_weights"] for c in range(NCORES)], 0)
    usage = np.concatenate([outs[c]["o_usage"] for c in range(NCORES)], 0)
    link = np.concatenate([outs[c]["o_link"] for c in range(NCORES)], 0)
    precedence = np.concatenate([outs[c]["o_precedence"] for c in range(NCORES)], 0)
    return (read_weights, write_weights, usage, link, precedence)
